# revision 1
# baseline (speedup 1.0000x reference)
# Trainium2 Bass kernel for nn_DecoderBlock (B=4, T=2048, E=1024, H=16, D=64, FF=4096).
#
# Sharding: 8-way data parallel, zero collectives. Core c = 2*b + h handles batch b
# and the interleaved half of the sequence: 128-row q-blocks {2s+h : s=0..7}
# (1024 q rows per core). K/V are computed per-core for the full T=2048 rows of its
# batch (duplicated across the two cores of a batch pair) so attention needs no
# cross-core communication. The interleaved block assignment makes the causal
# work pattern identical on every core (uniform SPMD program): q-slot s statically
# attends keys [0, 256*(s+1)), with a per-core {0,1} multiplicative mask (input
# data) handling the parity-dependent diagonal.
#
# On-chip layout: activations flow feature-major (S^T = [k, q]) through attention so
# softmax needs no transposes of the probability matrix. Softmax uses no max
# subtraction (scores are ~N(0, 0.25^2) by construction); 1/denominator is
# exp(-ln(d)) so the whole kernel uses one ACT table set (exp/ln/relu).
# Matmuls run in bf16 with fp32 PSUM accumulation; LN statistics, residuals and
# the final output stay fp32. LN gains (g1,g2) and the attention 1/sqrt(E) scale
# are folded into the weights on the host; beta terms become per-feature biases.

import numpy as np
import ml_dtypes
from contextlib import ExitStack

BF16 = ml_dtypes.bfloat16

B, T, E, H, D, FF = 4, 2048, 1024, 16, 64, 4096
M = 1024          # q rows per core
NCORES = 8
NS = 8            # q slots (128 rows) per core
ET = E // 128     # 8 e-tiles
TK = T // 128     # 16 k-tiles
FT = FF // 128    # 32 ff-tiles
NP = H // 2       # 8 head pairs
EPS = 1e-5

_CACHE = {}


def _build(repeat=1):
    """Build (and cache) the Bass module for one core's uniform program.

    repeat>1 emits the whole body N times (identical I/O) — used only for
    slope-based wall-clock timing of one body on hardware.
    """
    key = ("nc", repeat)
    if key in _CACHE:
        return _CACHE[key]

    import concourse.bacc as bacc
    import concourse.tile as tile
    import concourse.mybir as mybir
    from concourse import masks as cmasks

    dt = mybir.dt
    f32, bf16 = dt.float32, dt.bfloat16
    AF = mybir.ActivationFunctionType
    OP = mybir.AluOpType

    nc = bacc.Bacc("TRN2", target_bir_lowering=False, debug=False,
                   num_devices=NCORES)

    # Every activation we use (Exp, Ln, Relu, Copy, Identity) lives in the
    # 'natural_log_exp_and_others' table set. The default per-function set
    # choice alternates home sets (exp_and_others vs natural_log), inserting
    # ~80 ACT table loads (~100us). Restrict the chooser to the one set that
    # covers everything -> a single load.
    import types
    import bass_rust as _br

    def _insert_act_loads_one_set(self):
        has_activation = any(
            isinstance(i, mybir.InstActivation)
            for b in self.main_func.blocks for i in b.instructions)
        if not has_activation:
            return
        tabs = bacc.get_activation_tables(self.m.arch)
        ours = {mybir.ActivationFunctionType.Exp, mybir.ActivationFunctionType.Ln,
                mybir.ActivationFunctionType.Relu, mybir.ActivationFunctionType.Copy,
                mybir.ActivationFunctionType.Identity}
        filt = []
        for name, fns in tabs.items():
            if name == "natural_log_exp_and_others":
                assert ours <= fns
                filt.append((name, fns))
            else:
                filt.append((name, fns - ours))
        _br.insert_act_table_loads(self, filt)

    nc.insert_act_table_loads = types.MethodType(_insert_act_loads_one_set, nc)

    # ----- DRAM I/O -----
    x_full = nc.dram_tensor("x_full", [T, E], f32, kind="ExternalInput").ap()
    x_q = nc.dram_tensor("x_q", [M, E], f32, kind="ExternalInput").ap()
    xqp = nc.dram_tensor("xqp", [M, E], f32, kind="ExternalInput").ap()
    # weights arrive pre-arranged on the host into SBUF layout
    # [128 partitions, <tile dims>] so each DMA is one long contiguous run
    # per partition (minimal descriptor count).
    wq = nc.dram_tensor("wq", [128, ET * H * D], bf16, kind="ExternalInput").ap()
    wk = nc.dram_tensor("wk", [128, ET * H * D], bf16, kind="ExternalInput").ap()
    wv = nc.dram_tensor("wv", [128, ET * H * D], bf16, kind="ExternalInput").ap()
    projw = nc.dram_tensor("projw", [128, NP * E], bf16, kind="ExternalInput").ap()
    w1 = nc.dram_tensor("w1", [128, FT * ET * 128], bf16,
                        kind="ExternalInput").ap()
    w2 = nc.dram_tensor("w2", [128, FT * E], bf16, kind="ExternalInput").ap()
    qb_d = nc.dram_tensor("qb", [128, ET], f32, kind="ExternalInput").ap()
    kb_d = nc.dram_tensor("kb", [128, ET], f32, kind="ExternalInput").ap()
    vb_d = nc.dram_tensor("vb", [128, H * D], f32, kind="ExternalInput").ap()
    b1_d = nc.dram_tensor("b1", [128, FT], f32, kind="ExternalInput").ap()
    bf2_d = nc.dram_tensor("bf2b", [128, E], f32, kind="ExternalInput").ap()
    maskE_d = nc.dram_tensor("maskE", [128, 256], bf16, kind="ExternalInput").ap()
    maskO_d = nc.dram_tensor("maskO", [128, 256], bf16, kind="ExternalInput").ap()
    out = nc.dram_tensor("out", [M, E], f32, kind="ExternalOutput").ap()

    with tile.TileContext(nc) as tc:
      for _rep in range(repeat):
        es = ExitStack()
        with es:
            # ---------- constants (whole kernel) ----------
            constp = es.enter_context(tc.tile_pool(name="const", bufs=1))
            ident = constp.tile([128, 128], bf16)
            cmasks.make_identity(nc, ident[:])
            ones64 = constp.tile([128, 64], bf16)
            nc.gpsimd.memset(ones64[:], 1.0)
            maskE = constp.tile([128, 256], bf16)
            nc.sync.dma_start(maskE[:], maskE_d)
            maskO = constp.tile([128, 256], bf16)
            nc.sync.dma_start(maskO[:], maskO_d)
            qb = constp.tile([128, ET], f32)
            nc.sync.dma_start(qb[:], qb_d)
            kb = constp.tile([128, ET], f32)
            nc.sync.dma_start(kb[:], kb_d)
            vb = constp.tile([128, H * D], f32)
            nc.sync.dma_start(vb[:], vb_d)
            b1 = constp.tile([128, FT], f32)
            nc.sync.dma_start(b1[:], b1_d)
            bf2 = constp.tile([128, E], f32)
            nc.sync.dma_start(bf2[:], bf2_d)
            eps_t = constp.tile([128, 1], f32)
            nc.gpsimd.memset(eps_t[:], EPS)

            # helper: layernorm one 128-row chunk (fp32 src slice in SBUF) and
            # write the transposed bf16 result into dst_T[:, et, col:col+128].
            def ln_stats(src, statp):
                st = statp.tile([128, 2, 6], f32, tag="st")
                for g in range(2):
                    nc.vector.bn_stats(st[:, g, :], src[:, g * 512:(g + 1) * 512])
                ag = statp.tile([128, 2], f32, tag="ag")
                nc.vector.bn_aggr(ag[:], st[:])
                lv = statp.tile([128, 1], f32, tag="lv")
                nc.scalar.activation(lv[:], ag[:, 1:2], AF.Ln, bias=eps_t[:])
                rstd = statp.tile([128, 1], f32, tag="rstd")
                nc.scalar.activation(rstd[:], lv[:], AF.Exp, scale=-0.5)
                return ag, rstd

            def ln_chunk(src, dst_T, col, statp, lnstage, tpsum, ci,
                         stats=None):
                ag, rstd = stats if stats is not None else ln_stats(src, statp)
                lc = lnstage.tile([128, E], bf16)
                # split the normalize-apply across DVE and GPSIMD so the
                # per-chunk chain latency halves and both engines share work
                nc.vector.tensor_scalar(lc[:, 0:512], src[:, 0:512],
                                        ag[:, 0:1], rstd[:],
                                        OP.subtract, OP.mult)
                nc.gpsimd.tensor_scalar(lc[:, 512:1024], src[:, 512:1024],
                                        ag[:, 0:1], rstd[:],
                                        OP.subtract, OP.mult)
                import os as _os
                if _os.environ.get("KT_DMA_TRANSPOSE", "0") == "1":
                    for et in range(ET):
                        nc.sync.dma_start(dst_T[:, et, col:col + 128],
                                            lc[:, et * 128:(et + 1) * 128],
                                            transpose=True)
                else:
                    for et in range(ET):
                        tp = tpsum.tile([128, 128], bf16)
                        nc.tensor.transpose(tp[:],
                                            lc[:, et * 128:(et + 1) * 128],
                                            ident[:])
                        dst = dst_T[:, et, col:col + 128]
                        if (et + ci) % 2 == 0:
                            nc.vector.tensor_copy(dst, tp[:])
                        else:
                            nc.scalar.copy(dst, tp[:])
                        # (copies stay off GPSIMD: it cannot read PSUM)

            # ---------- scope B: qT/kT/v (strict stack nesting) ----------
            xmid = es.enter_context(tc.tile_pool(name="xmidp", bufs=1)).tile(
                [128, NS, E], f32)
            with ExitStack() as sB:
                qT = sB.enter_context(tc.tile_pool(name="qTp", bufs=1)).tile(
                    [128, NP, M], bf16)
                kT = sB.enter_context(tc.tile_pool(name="kTp", bufs=1)).tile(
                    [128, NP, T], bf16)
                vS = sB.enter_context(tc.tile_pool(name="vp", bufs=1)).tile(
                    [128, TK, H * D], bf16)

                # ---------- scope A: LN1 + QKV projections ----------
                with ExitStack() as sA:
                    wpool = sA.enter_context(tc.tile_pool(name="wpool", bufs=1))
                    stage = sA.enter_context(tc.tile_pool(name="xstage", bufs=3))
                    lnstage = sA.enter_context(tc.tile_pool(name="lnstage", bufs=3))
                    statp = sA.enter_context(tc.tile_pool(name="statp", bufs=6))
                    tpsum = sA.enter_context(
                        tc.tile_pool(name="tpsum", bufs=4, space="PSUM"))
                    qps = sA.enter_context(
                        tc.tile_pool(name="qps", bufs=2, space="PSUM"))

                    with tc.tile_pool(name="lnqp", bufs=1) as lnqp:
                        lnq = lnqp.tile([128, ET, M], bf16)
                        # LN1 on the gathered q rows -> lnq (e-major)
                        for s in range(NS):
                            xc = stage.tile([128, E], f32)
                            nc.scalar.dma_start(xc[:], x_q[s * 128:(s + 1) * 128, :])
                            ln_chunk(xc[:], lnq, s * 128, statp, lnstage, tpsum, s)

                        # pre-issue DMA+stats of the first x_full chunks so
                        # their normalize-applies are ready right after Q^T
                        pre = []
                        for c in range(2):
                            xc = stage.tile([128, E], f32)
                            nc.scalar.dma_start(
                                xc[:], x_full[c * 128:(c + 1) * 128, :])
                            pre.append((xc, ln_stats(xc[:], statp)))

                        # Q^T = (wq)^T @ lnq^T   [hd, q]
                        wq_sb = wpool.tile([128, ET, H * D], bf16, tag="w")
                        nc.sync.dma_start(wq_sb[:], wq.rearrange(
                            "p (et n) -> p et n", et=ET))
                        for m in range(ET):
                            ps = qps.tile([128, 1024], f32)
                            for qc in range(2):
                                for et in range(ET):
                                    nc.tensor.matmul(
                                        ps[:, qc * 512:(qc + 1) * 512],
                                        lhsT=wq_sb[:, et, m * 128:(m + 1) * 128],
                                        rhs=lnq[:, et, qc * 512:(qc + 1) * 512],
                                        start=(et == 0), stop=(et == ET - 1))
                            nc.vector.tensor_scalar_add(
                                qT[:, m, :], ps[:], qb[:, m:m + 1])

                    lnf = sA.enter_context(tc.tile_pool(name="lnfp", bufs=1)).tile(
                        [128, ET, T], bf16)
                    # LN1 on the full batch rows -> lnf
                    for c in range(TK):
                        if c < len(pre):
                            xc, stats = pre[c]
                            ln_chunk(xc[:], lnf, c * 128, statp, lnstage,
                                     tpsum, c, stats=stats)
                        else:
                            xc = stage.tile([128, E], f32)
                            nc.scalar.dma_start(
                                xc[:], x_full[c * 128:(c + 1) * 128, :])
                            ln_chunk(xc[:], lnf, c * 128, statp, lnstage,
                                     tpsum, c)

                    # K^T [hd, t]
                    wk_sb = wpool.tile([128, ET, H * D], bf16, tag="w")
                    nc.sync.dma_start(wk_sb[:], wk.rearrange(
                        "p (et n) -> p et n", et=ET))
                    for m in range(ET):
                        for kh in range(2):
                            ps = qps.tile([128, 1024], f32)
                            for kc in range(2 * kh, 2 * kh + 2):
                                for et in range(ET):
                                    nc.tensor.matmul(
                                        ps[:, (kc % 2) * 512:(kc % 2 + 1) * 512],
                                        lhsT=wk_sb[:, et, m * 128:(m + 1) * 128],
                                        rhs=lnf[:, et, kc * 512:(kc + 1) * 512],
                                        start=(et == 0), stop=(et == ET - 1))
                            nc.vector.tensor_scalar_add(
                                kT[:, m, kh * 1024:(kh + 1) * 1024], ps[:],
                                kb[:, m:m + 1])

                    # V [t, hd] (token-major)
                    wv_sb = wpool.tile([128, ET, H * D], bf16, tag="w")
                    nc.sync.dma_start(wv_sb[:], wv.rearrange(
                        "p (et n) -> p et n", et=ET))
                    for t in range(TK):
                        ps = qps.tile([128, 1024], f32)
                        for hc in range(2):
                            for et in range(ET):
                                nc.tensor.matmul(
                                    ps[:, hc * 512:(hc + 1) * 512],
                                    lhsT=lnf[:, et, t * 128:(t + 1) * 128],
                                    rhs=wv_sb[:, et, hc * 512:(hc + 1) * 512],
                                    start=(et == 0), stop=(et == ET - 1))
                        nc.vector.tensor_add(vS[:, t, :], ps[:], vb[:])

                # ---------- attention + output projection ----------
                with ExitStack() as sC:
                    oT = sC.enter_context(tc.tile_pool(name="oTp", bufs=1)).tile(
                        [128, NP, M], bf16)

                    with ExitStack() as sAtt:
                        ptp = sAtt.enter_context(tc.tile_pool(name="ptp", bufs=6))
                        normp = sAtt.enter_context(
                            tc.tile_pool(name="normp", bufs=3))
                        apsum = sAtt.enter_context(
                            tc.tile_pool(name="apsum", bufs=1, space="PSUM"))
                        spsum = sAtt.enter_context(
                            tc.tile_pool(name="spsum", bufs=2, space="PSUM"))

                        maskEv = maskE[:].rearrange("p (h q) -> p h q", h=2)
                        maskOv = maskO[:].rearrange("p (h q) -> p h q", h=2)

                        for p in range(NP):
                            av = apsum.tile([128, M], f32, tag="av")
                            den = apsum.tile([128, M], f32, tag="den")
                            pend = []  # software-pipelined AV work

                            def do_av(item, av=av, den=den, p=p):
                                half, kt, qlo, pt = item
                                colr = slice(512 * half + qlo, 512 * (half + 1))
                                st = (kt == 0)
                                sp = (kt == (7 if half == 0 else 15))
                                for h in range(2):
                                    hd = (2 * p + h) * 64
                                    nc.tensor.matmul(
                                        av[64 * h:64 * h + 64, colr],
                                        lhsT=vS[:, kt, hd:hd + 64],
                                        rhs=pt[:, h, qlo:512],
                                        start=st, stop=sp, skip_group_check=True)
                                    nc.tensor.matmul(
                                        den[64 * h:64 * h + 64, colr],
                                        lhsT=ones64[:],
                                        rhs=pt[:, h, qlo:512],
                                        start=st, stop=sp, skip_group_check=True)

                            for half in range(2):
                                for kt in range(8 if half == 0 else 16):
                                    qlo = max(0, 128 * (kt // 2) - 512 * half)
                                    ps = spsum.tile([128, 2, 512], f32)
                                    for h in range(2):
                                        nc.tensor.matmul(
                                            ps[:, h, qlo:512],
                                            lhsT=kT[64 * h:64 * h + 64, p,
                                                    kt * 128:(kt + 1) * 128],
                                            rhs=qT[64 * h:64 * h + 64, p,
                                                   512 * half + qlo:
                                                   512 * (half + 1)],
                                            start=True, stop=True)
                                    pt = ptp.tile([128, 2, 512], bf16)
                                    nc.scalar.activation(
                                        pt[:, :, qlo:512], ps[:, :, qlo:512],
                                        AF.Exp)
                                    if (kt // 2) >= 4 * half:
                                        mk = maskEv if kt % 2 == 0 else maskOv
                                        nc.vector.tensor_mul(
                                            pt[:, :, qlo:qlo + 128],
                                            pt[:, :, qlo:qlo + 128], mk)
                                    pend.append((half, kt, qlo, pt))
                                    if len(pend) > 4:
                                        do_av(pend.pop(0))
                            for item in pend:
                                do_av(item)
                            tln = normp.tile([128, M], f32, tag="tln")
                            nc.scalar.activation(tln[:], den[:], AF.Ln)
                            rcp = normp.tile([128, M], bf16, tag="rcp")
                            nc.scalar.activation(rcp[:], tln[:], AF.Exp,
                                                 scale=-1.0)
                            nc.vector.tensor_mul(oT[:, p, :], av[:], rcp[:])

                    # output projection + residual -> xmid (fp32)
                    with ExitStack() as sProj:
                        xqps = sProj.enter_context(
                            tc.tile_pool(name="xqpp", bufs=1)).tile(
                            [128, NS, E], f32)
                        for s in range(NS):
                            nc.sync.dma_start(
                                xqps[:, s, :], xqp[s * 128:(s + 1) * 128, :])
                        pw_sb = sProj.enter_context(
                            tc.tile_pool(name="pwp", bufs=1)).tile(
                            [128, NP, E], bf16)
                        nc.sync.dma_start(pw_sb[:], projw.rearrange(
                            "p (m e) -> p m e", m=NP))
                        pps = sProj.enter_context(
                            tc.tile_pool(name="pps", bufs=3, space="PSUM"))
                        for qm in range(NS):
                            ps = pps.tile([128, 1024], f32)
                            for ec in range(2):
                                for pk in range(NP):
                                    nc.tensor.matmul(
                                        ps[:, ec * 512:(ec + 1) * 512],
                                        lhsT=oT[:, pk, qm * 128:(qm + 1) * 128],
                                        rhs=pw_sb[:, pk, ec * 512:(ec + 1) * 512],
                                        start=(pk == 0), stop=(pk == NP - 1))
                            nc.vector.tensor_add(
                                xmid[:, qm, :], ps[:], xqps[:, qm, :])

            # ---------- scope D: LN2 + FFN ----------
            with ExitStack() as sD:
                ln2T = sD.enter_context(tc.tile_pool(name="ln2p", bufs=1)).tile(
                    [128, ET, M], bf16)
                w2_sb = sD.enter_context(tc.tile_pool(name="w2p", bufs=1)).tile(
                    [128, FT, E], bf16)
                nc.sync.dma_start(w2_sb[:], w2.rearrange(
                    "p (ft e) -> p ft e", ft=FT))

                with ExitStack() as sLN2:
                    statp2 = sLN2.enter_context(tc.tile_pool(name="statp2", bufs=6))
                    lnstage2 = sLN2.enter_context(
                        tc.tile_pool(name="lnstage2", bufs=3))
                    tpsum2 = sLN2.enter_context(
                        tc.tile_pool(name="tpsum2", bufs=3, space="PSUM"))
                    for qm in range(NS):
                        ln_chunk(xmid[:, qm, :], ln2T, qm * 128, statp2,
                                 lnstage2, tpsum2, qm)
                        # after LN2 consumed xmid, fold the final bf2 bias in
                        nc.vector.tensor_add(xmid[:, qm, :], xmid[:, qm, :],
                                             bf2[:])

                rtp = sD.enter_context(tc.tile_pool(name="rtp", bufs=1))
                w1p = sD.enter_context(tc.tile_pool(name="w1p", bufs=4))
                zps = sD.enter_context(
                    tc.tile_pool(name="zps", bufs=2, space="PSUM"))
                ops = sD.enter_context(
                    tc.tile_pool(name="ops", bufs=2, space="PSUM"))
                outp = sD.enter_context(tc.tile_pool(name="outp", bufs=3))

                for half in range(2):
                    rT = rtp.tile([128, FT, 512], bf16, tag="rT")
                    for fm in range(FT):
                        w1f = w1p.tile([128, ET, 128], bf16)
                        nc.sync.dma_start(
                            w1f[:], w1.rearrange("p (fm et f) -> p fm et f",
                                                 fm=FT, et=ET)[:, fm])
                        zp = zps.tile([128, 512], f32)
                        for et in range(ET):
                            nc.tensor.matmul(
                                zp[:],
                                lhsT=w1f[:, et, :],
                                rhs=ln2T[:, et, half * 512:(half + 1) * 512],
                                start=(et == 0), stop=(et == ET - 1))
                        nc.scalar.activation(rT[:, fm, :], zp[:], AF.Relu,
                                             bias=b1[:, fm:fm + 1])
                    for qq in range(4):
                        qm = half * 4 + qq
                        ot = outp.tile([128, E], f32)
                        op = ops.tile([128, 1024], f32)
                        for ec in range(2):
                            for fk in range(FT):
                                nc.tensor.matmul(
                                    op[:, ec * 512:(ec + 1) * 512],
                                    lhsT=rT[:, fk, qq * 128:(qq + 1) * 128],
                                    rhs=w2_sb[:, fk, ec * 512:(ec + 1) * 512],
                                    start=(fk == 0), stop=(fk == FT - 1))
                        nc.vector.tensor_add(ot[:], op[:], xmid[:, qm, :])
                        nc.scalar.dma_start(out[qm * 128:(qm + 1) * 128, :], ot[:])

    nc.compile()
    _CACHE[key] = nc
    return nc


def _prep_inputs(x, wq, wk, wv, proj_w, proj_b, g1, beta1, g2, beta2, w1, bf1,
                 w2, bf2):
    """Host-side sharding + weight folding. Returns list of 8 in_maps."""
    f32 = np.float32
    x = np.asarray(x, f32)
    scale = float(E) ** -0.5

    Wq = np.asarray(wq, f32).transpose(1, 0, 2).reshape(E, H * D) * scale
    Wk = np.asarray(wk, f32).transpose(1, 0, 2).reshape(E, H * D)
    Wv = np.asarray(wv, f32).transpose(1, 0, 2).reshape(E, H * D)
    g1 = np.asarray(g1, f32)
    beta1 = np.asarray(beta1, f32)
    g2 = np.asarray(g2, f32)
    beta2 = np.asarray(beta2, f32)
    w1 = np.asarray(w1, f32)
    w2 = np.asarray(w2, f32)
    bf1 = np.asarray(bf1, f32)
    bf2 = np.asarray(bf2, f32)
    proj_w = np.asarray(proj_w, f32)
    proj_b = np.asarray(proj_b, f32)

    def sb_layout(w, ntile):
        # [ntile*128, N] -> [128, ntile*N] with per-partition contiguous tiles
        n = w.shape[1]
        return np.ascontiguousarray(
            w.reshape(ntile, 128, n).transpose(1, 0, 2).reshape(128, ntile * n))

    wq_b = sb_layout((Wq * g1[:, None]).astype(BF16), ET)
    wk_b = sb_layout((Wk * g1[:, None]).astype(BF16), ET)
    wv_b = sb_layout((Wv * g1[:, None]).astype(BF16), ET)
    qbias = beta1 @ Wq
    kbias = beta1 @ Wk
    vbias = beta1 @ Wv
    w1_b = np.ascontiguousarray(
        (w1 * g2[:, None]).astype(BF16)
        .reshape(ET, 128, FT, 128).transpose(1, 2, 0, 3)
        .reshape(128, FT * ET * 128))
    b1v = bf1 + beta2 @ w1
    w2_b = sb_layout(w2.astype(BF16), FT)
    projw_b = sb_layout(proj_w.astype(BF16), NP)

    qb = np.ascontiguousarray(qbias.reshape(ET, 128).T, f32)
    kb = np.ascontiguousarray(kbias.reshape(ET, 128).T, f32)
    vb = np.ascontiguousarray(np.broadcast_to(vbias, (128, H * D)), f32)
    b1m = np.ascontiguousarray(b1v.reshape(FT, 128).T, f32)
    bf2m = np.ascontiguousarray(np.broadcast_to(bf2, (128, E)), f32)

    tri = np.triu(np.ones((128, 128), f32))  # [k_row, q_col]: 1 iff k <= q
    onesm = np.ones((128, 128), f32)
    zerosm = np.zeros((128, 128), f32)
    mE = {0: tri, 1: onesm}
    mO = {0: zerosm, 1: tri}

    in_maps = []
    for c in range(NCORES):
        b, hpar = c // 2, c % 2
        xb = x[b]
        xq = np.ascontiguousarray(
            xb.reshape(TK, 128, E)[hpar::2].reshape(M, E), f32)
        in_maps.append({
            "x_full": np.ascontiguousarray(xb, f32),
            "x_q": xq,
            "xqp": xq + proj_b[None, :].astype(f32),
            "wq": wq_b, "wk": wk_b, "wv": wv_b,
            "projw": projw_b, "w1": w1_b, "w2": w2_b,
            "qb": qb, "kb": kb, "vb": vb, "b1": b1m, "bf2b": bf2m,
            "maskE": np.ascontiguousarray(
                np.tile(mE[hpar], (1, 2))).astype(BF16),
            "maskO": np.ascontiguousarray(
                np.tile(mO[hpar], (1, 2))).astype(BF16),
        })
    return in_maps


def _run(inputs, trace=False):
    from concourse.bass_utils import run_bass_kernel_spmd
    nc = _build()
    in_maps = _prep_inputs(**inputs)
    res = run_bass_kernel_spmd(nc, in_maps, core_ids=list(range(NCORES)),
                               trace=trace)
    full = np.empty((B, T, E), np.float32)
    for c in range(NCORES):
        b, hpar = c // 2, c % 2
        full[b].reshape(TK, 128, E)[hpar::2] = (
            res.results[c]["out"].reshape(NS, 128, E))
    return full, res


def kernel(**inputs) -> np.ndarray:
    out, _ = _run(inputs, trace=False)
    return out



# revision 13
# speedup vs baseline: 1.0630x; 1.0630x over previous
# Trainium2 Bass kernel for nn_DecoderBlock (B=4, T=2048, E=1024, H=16, D=64, FF=4096).
#
# Sharding: 8-way data parallel, zero collectives. Core c = 2*b + h handles batch b
# and the interleaved half of the sequence. The host PERMUTES the batch rows so
# that the core's own 1024 q rows (global 128-blocks {2s+h}) come first
# (physical blocks 0-7) and the other parity's blocks follow (8-15). With this
# ordering the program is uniform SPMD: q-block s statically attends own-parity
# key tiles 0..s (triangular mask on tile s) and other-parity tiles 8..8+s,
# where tile 8+s is all-or-nothing by parity (input 0/1 mask).
#
# Attention uses a transposed AV: per (head, q-block) the matmul computes
# out[q(128), d(64)+1] with a fused ones-column appended to V producing the
# softmax denominator for free; normalization is then a native per-partition
# tensor_scalar multiply by 1/den. This makes the AV+den tensor-engine cost
# proportional to 65 output columns instead of the 512-wide probability tiles.
#
# On-chip layout: LN1 is computed once over the permuted batch (feature-major
# lnf via PE transposes); Q reads the first 1024 columns, K/V the full 2048.
# Softmax uses no max subtraction (scores ~N(0, 0.25^2)); matmuls run in bf16
# with fp32 PSUM accumulation. LN gains and the attention 1/sqrt(E) scale are
# folded into the weights on the host.

import numpy as np
import ml_dtypes
from contextlib import ExitStack

BF16 = ml_dtypes.bfloat16

B, T, E, H, D, FF = 4, 2048, 1024, 16, 64, 4096
M = 1024          # q rows per core
NCORES = 8
NS = 8            # q slots (128 rows) per core
ET = E // 128     # 8 e-tiles
TK = T // 128     # 16 k-tiles
FT = FF // 128    # 32 ff-tiles
NP = H // 2       # 8 head pairs
EPS = 1e-5

_CACHE = {}


def _build(repeat=1):
    """Build (and cache) the Bass module for one core's uniform program."""
    key = ("nc", repeat)
    if key in _CACHE:
        return _CACHE[key]

    import concourse.bacc as bacc
    import concourse.tile as tile
    import concourse.mybir as mybir
    from concourse import masks as cmasks

    dt = mybir.dt
    f32, bf16 = dt.float32, dt.bfloat16
    AF = mybir.ActivationFunctionType
    OP = mybir.AluOpType

    nc = bacc.Bacc("TRN2", target_bir_lowering=False, debug=False,
                   num_devices=NCORES)

    # Keep every activation (Exp, Ln, Relu, Copy, Identity) in the single
    # 'natural_log_exp_and_others' table set -> one ACT table load.
    import types
    import bass_rust as _br

    def _insert_act_loads_one_set(self):
        has_activation = any(
            isinstance(i, mybir.InstActivation)
            for b in self.main_func.blocks for i in b.instructions)
        if not has_activation:
            return
        tabs = bacc.get_activation_tables(self.m.arch)
        ours = {mybir.ActivationFunctionType.Exp, mybir.ActivationFunctionType.Ln,
                mybir.ActivationFunctionType.Relu, mybir.ActivationFunctionType.Copy,
                mybir.ActivationFunctionType.Identity}
        filt = []
        for name, fns in tabs.items():
            if name == "natural_log_exp_and_others":
                assert ours <= fns
                filt.append((name, fns))
            else:
                filt.append((name, fns - ours))
        _br.insert_act_table_loads(self, filt)

    nc.insert_act_table_loads = types.MethodType(_insert_act_loads_one_set, nc)

    # ----- DRAM I/O -----
    # x_full rows are host-permuted: own-parity 128-blocks first (q rows
    # 0..1023), other-parity blocks after (rows 1024..2047).
    x_full = nc.dram_tensor("x_full", [T, E], f32, kind="ExternalInput").ap()
    xqp = nc.dram_tensor("xqp", [M, E], f32, kind="ExternalInput").ap()
    wq = nc.dram_tensor("wq", [128, ET * H * D], bf16, kind="ExternalInput").ap()
    wk = nc.dram_tensor("wk", [128, ET * H * D], bf16, kind="ExternalInput").ap()
    wv = nc.dram_tensor("wv", [128, ET * H * D], bf16, kind="ExternalInput").ap()
    projw = nc.dram_tensor("projw", [128, NP * E], bf16, kind="ExternalInput").ap()
    w1 = nc.dram_tensor("w1", [128, FT * ET * 128], bf16,
                        kind="ExternalInput").ap()
    w2 = nc.dram_tensor("w2", [128, FT * E], bf16, kind="ExternalInput").ap()
    qb_d = nc.dram_tensor("qb", [128, ET], f32, kind="ExternalInput").ap()
    kb_d = nc.dram_tensor("kb", [128, ET], f32, kind="ExternalInput").ap()
    vb_d = nc.dram_tensor("vb", [128, H * D], f32, kind="ExternalInput").ap()
    b1_d = nc.dram_tensor("b1", [128, FT], f32, kind="ExternalInput").ap()
    bf2_d = nc.dram_tensor("bf2b", [128, E], f32, kind="ExternalInput").ap()
    # maskT: triangular (k<=q) mask for the own-parity diagonal tile, tiled x2
    # for both heads. maskP: 0/1 by parity for the other-parity companion tile.
    maskT_d = nc.dram_tensor("maskT", [128, 256], bf16, kind="ExternalInput").ap()
    maskP_d = nc.dram_tensor("maskP", [128, 256], bf16, kind="ExternalInput").ap()
    out = nc.dram_tensor("out", [M, E], f32, kind="ExternalOutput").ap()

    with tile.TileContext(nc) as tc:
      for _rep in range(repeat):
        es = ExitStack()
        with es:
            # ---------- constants (whole kernel) ----------
            constp = es.enter_context(tc.tile_pool(name="const", bufs=1))
            ident = constp.tile([128, 128], bf16)
            cmasks.make_identity(nc, ident[:])
            maskT = constp.tile([128, 256], bf16)
            nc.sync.dma_start(maskT[:], maskT_d)
            maskP = constp.tile([128, 256], bf16)
            nc.sync.dma_start(maskP[:], maskP_d)
            qb = constp.tile([128, ET], f32)
            nc.sync.dma_start(qb[:], qb_d)
            kb = constp.tile([128, ET], f32)
            nc.sync.dma_start(kb[:], kb_d)
            vb = constp.tile([128, H * D], f32)
            nc.sync.dma_start(vb[:], vb_d)
            b1 = constp.tile([128, FT], f32)
            nc.sync.dma_start(b1[:], b1_d)
            bf2 = constp.tile([128, E], f32)
            nc.sync.dma_start(bf2[:], bf2_d)
            eps_t = constp.tile([128, 1], f32)
            nc.gpsimd.memset(eps_t[:], EPS)
            zer260 = constp.tile([128, 260], bf16)
            nc.gpsimd.memset(zer260[:], 0.0)

            def ln_stats(src, statp):
                st = statp.tile([128, 2, 6], f32, tag="st")
                for g in range(2):
                    nc.vector.bn_stats(st[:, g, :], src[:, g * 512:(g + 1) * 512])
                ag = statp.tile([128, 2], f32, tag="ag")
                nc.vector.bn_aggr(ag[:], st[:])
                lv = statp.tile([128, 1], f32, tag="lv")
                nc.scalar.activation(lv[:], ag[:, 1:2], AF.Ln, bias=eps_t[:])
                rstd = statp.tile([128, 1], f32, tag="rstd")
                nc.scalar.activation(rstd[:], lv[:], AF.Exp, scale=-0.5)
                return ag, rstd

            def ln_chunk(src, dst_T, col, statp, lnstage, tpsum, ci,
                         stats=None):
                ag, rstd = stats if stats is not None else ln_stats(src, statp)
                lc = lnstage.tile([128, E], bf16)
                nc.vector.tensor_scalar(lc[:, 0:512], src[:, 0:512],
                                        ag[:, 0:1], rstd[:],
                                        OP.subtract, OP.mult)
                nc.gpsimd.tensor_scalar(lc[:, 512:1024], src[:, 512:1024],
                                        ag[:, 0:1], rstd[:],
                                        OP.subtract, OP.mult)
                for et in range(ET):
                    tp = tpsum.tile([128, 128], bf16)
                    nc.tensor.transpose(tp[:],
                                        lc[:, et * 128:(et + 1) * 128],
                                        ident[:])
                    dst = dst_T[:, et, col:col + 128]
                    if (et + ci) % 2 == 0:
                        nc.vector.tensor_copy(dst, tp[:])
                    else:
                        nc.scalar.copy(dst, tp[:])

            # ---------- scope B: qT/kT/v ----------
            xmid = es.enter_context(tc.tile_pool(name="xmidp", bufs=1)).tile(
                [128, NS, E], f32)
            with ExitStack() as sB:
                qT = sB.enter_context(tc.tile_pool(name="qTp", bufs=1)).tile(
                    [128, NP, M], bf16)
                kT = sB.enter_context(tc.tile_pool(name="kTp", bufs=1)).tile(
                    [128, NP, T], bf16)
                # V with a fused ones-column per head: per head pair p and
                # head h the 65 columns are [64 d | 1].
                vS = sB.enter_context(tc.tile_pool(name="vp", bufs=1)).tile(
                    [128, TK, NP, 2, 65], bf16)
                nc.gpsimd.memset(vS[:], 1.0)

                # ---------- scope A: LN1 + QKV projections ----------
                with ExitStack() as sA:
                    wpool = sA.enter_context(tc.tile_pool(name="wpool", bufs=1))
                    stage = sA.enter_context(tc.tile_pool(name="xstage", bufs=3))
                    lnstage = sA.enter_context(tc.tile_pool(name="lnstage", bufs=3))
                    statp = sA.enter_context(tc.tile_pool(name="statp", bufs=6))
                    tpsum = sA.enter_context(
                        tc.tile_pool(name="tpsum", bufs=4, space="PSUM"))
                    qps = sA.enter_context(
                        tc.tile_pool(name="qps", bufs=2, space="PSUM"))

                    lnf = sA.enter_context(tc.tile_pool(name="lnfp", bufs=1)).tile(
                        [128, ET, T], bf16)
                    # LN1 over the permuted batch; q rows are cols 0..1023.
                    for c in range(TK):
                        xc = stage.tile([128, E], f32)
                        nc.scalar.dma_start(
                            xc[:], x_full[c * 128:(c + 1) * 128, :])
                        ln_chunk(xc[:], lnf, c * 128, statp, lnstage,
                                 tpsum, c)

                    # Q^T [hd, q] from the first 1024 lnf columns
                    wq_sb = wpool.tile([128, ET, H * D], bf16, tag="w")
                    nc.sync.dma_start(wq_sb[:], wq.rearrange(
                        "p (et n) -> p et n", et=ET))
                    for m in range(ET):
                        ps = qps.tile([128, 1024], f32)
                        for qc in range(2):
                            for et in range(ET):
                                nc.tensor.matmul(
                                    ps[:, qc * 512:(qc + 1) * 512],
                                    lhsT=wq_sb[:, et, m * 128:(m + 1) * 128],
                                    rhs=lnf[:, et, qc * 512:(qc + 1) * 512],
                                    start=(et == 0), stop=(et == ET - 1))
                        nc.vector.tensor_scalar_add(
                            qT[:, m, :], ps[:], qb[:, m:m + 1])

                    # K^T [hd, t] over all 2048 columns
                    wk_sb = wpool.tile([128, ET, H * D], bf16, tag="w")
                    nc.sync.dma_start(wk_sb[:], wk.rearrange(
                        "p (et n) -> p et n", et=ET))
                    for m in range(ET):
                        for kh in range(2):
                            ps = qps.tile([128, 1024], f32)
                            for kc in range(2 * kh, 2 * kh + 2):
                                for et in range(ET):
                                    nc.tensor.matmul(
                                        ps[:, (kc % 2) * 512:(kc % 2 + 1) * 512],
                                        lhsT=wk_sb[:, et, m * 128:(m + 1) * 128],
                                        rhs=lnf[:, et, kc * 512:(kc + 1) * 512],
                                        start=(et == 0), stop=(et == ET - 1))
                            nc.vector.tensor_scalar_add(
                                kT[:, m, kh * 1024:(kh + 1) * 1024], ps[:],
                                kb[:, m:m + 1])

                    # V [t, hd] (token-major), written into the 65-col layout
                    wv_sb = wpool.tile([128, ET, H * D], bf16, tag="w")
                    nc.sync.dma_start(wv_sb[:], wv.rearrange(
                        "p (et n) -> p et n", et=ET))
                    for t in range(TK):
                        ps = qps.tile([128, 1024], f32)
                        for hc in range(2):
                            for et in range(ET):
                                nc.tensor.matmul(
                                    ps[:, hc * 512:(hc + 1) * 512],
                                    lhsT=lnf[:, et, t * 128:(t + 1) * 128],
                                    rhs=wv_sb[:, et, hc * 512:(hc + 1) * 512],
                                    start=(et == 0), stop=(et == ET - 1))
                        nc.vector.tensor_add(
                            vS[:, t, :, :, 0:64],
                            ps[:].rearrange("p (np two d) -> p np two d",
                                            np=NP, two=2),
                            vb[:].rearrange("p (np two d) -> p np two d",
                                            np=NP, two=2))

                # ---------- attention (transposed AV with fused den) ----------
                with ExitStack() as sC:
                    oT = sC.enter_context(tc.tile_pool(name="oTp", bufs=1)).tile(
                        [128, NP, M], bf16)
                    o_sb = sC.enter_context(tc.tile_pool(name="osbp", bufs=1)).tile(
                        [128, NS, NP, 2, 64], bf16)

                    with ExitStack() as sAtt:
                        ptp = sAtt.enter_context(tc.tile_pool(name="ptp", bufs=6))
                        rcpp = sAtt.enter_context(tc.tile_pool(name="rcpp", bufs=3))
                        oqps = sAtt.enter_context(
                            tc.tile_pool(name="oqps", bufs=1, space="PSUM"))
                        spsum = sAtt.enter_context(
                            tc.tile_pool(name="spsum", bufs=2, space="PSUM"))

                        maskTv = maskT[:].rearrange("p (h q) -> p h q", h=2)
                        maskPv = maskP[:].rearrange("p (h q) -> p h q", h=2)

                        for p in range(NP):
                            # oq[h]: [q(128) x (8 slots) x 65] accumulators
                            # [q(128) x (4 slots) x 65] accumulators, one per
                            # (head, s-half) to stay within 1 PSUM bank each
                            oq00 = oqps.tile([128, 4, 65], f32, tag="oq00")
                            oq01 = oqps.tile([128, 4, 65], f32, tag="oq01")
                            oq10 = oqps.tile([128, 4, 65], f32, tag="oq10")
                            oq11 = oqps.tile([128, 4, 65], f32, tag="oq11")
                            oq = [[oq00, oq01], [oq10, oq11]]
                            # one start=True zeroing matmul per tile: a
                            # start on any slot marks the whole 2KB bank
                            # pending-zero, so per-slot starts would wipe
                            # bank-mates mid-accumulation.
                            for tl in (oq00, oq01, oq10, oq11):
                                nc.tensor.matmul(
                                    tl[:].rearrange("p s c -> p (s c)"),
                                    lhsT=ident[:], rhs=zer260[:],
                                    start=True, stop=False,
                                    skip_group_check=True)
                            pend = []

                            def do_av(item, oq=oq, p=p):
                                kt, j, qlo, wid, pt = item
                                s0 = qlo // 128
                                for si in range(wid // 128):
                                    s = s0 + si
                                    if s < j:
                                        continue
                                    sp = (kt == 8 + s)
                                    for h in range(2):
                                        nc.tensor.matmul(
                                            oq[h][s // 4][:, s % 4, :],
                                            lhsT=pt[:, h,
                                                    si * 128:(si + 1) * 128],
                                            rhs=vS[:, kt, p, h, :],
                                            start=False, stop=sp,
                                            skip_group_check=True)

                            for kt in range(TK):
                                j = kt % 8   # diagonal q-block for this tile
                                qlo = 128 * j
                                # q-slices of <=512 covering [qlo, 1024)
                                starts = list(range(qlo, M, 512))
                                for qs in starts:
                                    wid = min(512, M - qs)
                                    ps = spsum.tile([128, 2, 512], f32)
                                    for h in range(2):
                                        nc.tensor.matmul(
                                            ps[:, h, 0:wid],
                                            lhsT=kT[64 * h:64 * h + 64, p,
                                                    kt * 128:(kt + 1) * 128],
                                            rhs=qT[64 * h:64 * h + 64, p,
                                                   qs:qs + wid],
                                            start=True, stop=True)
                                    pt = ptp.tile([128, 2, 512], bf16)
                                    nc.scalar.activation(
                                        pt[:, :, 0:wid], ps[:, :, 0:wid],
                                        AF.Exp)
                                    if qs == qlo:
                                        mk = maskTv if kt < 8 else maskPv
                                        nc.vector.tensor_mul(
                                            pt[:, :, 0:128],
                                            pt[:, :, 0:128], mk)
                                    pend.append((kt, j, qs, wid, pt))
                                    if len(pend) > 3:
                                        do_av(pend.pop(0))
                            for item in pend:
                                do_av(item)

                            # normalize: per-partition 1/den multiply
                            for h in range(2):
                                for sh in range(2):
                                    rcp = rcpp.tile([128, 4], f32, tag="rcp")
                                    nc.vector.reciprocal(
                                        rcp[:],
                                        oq[h][sh][:, :, 64:65].rearrange(
                                            "p s one -> p (s one)"))
                                    for si in range(4):
                                        s = sh * 4 + si
                                        nc.vector.tensor_scalar_mul(
                                            o_sb[:, s, p, h, :],
                                            oq[h][sh][:, si, 0:64],
                                            rcp[:, si:si + 1])

                    # transpose o_sb [q, hd] -> oT [hd, q] (attention PSUM
                    # pools are closed by now)
                    with ExitStack() as sTr:
                        tps2 = sTr.enter_context(
                            tc.tile_pool(name="tps2", bufs=4, space="PSUM"))
                        o_sbf = o_sb[:].rearrange("p s np two d -> p s (np two d)")
                        for s in range(NS):
                            for et in range(ET):
                                tp = tps2.tile([128, 128], bf16)
                                nc.tensor.transpose(
                                    tp[:], o_sbf[:, s, et * 128:(et + 1) * 128],
                                    ident[:])
                                dst = oT[:, et, s * 128:(s + 1) * 128]
                                if (et + s) % 2 == 0:
                                    nc.vector.tensor_copy(dst, tp[:])
                                else:
                                    nc.scalar.copy(dst, tp[:])

                    # output projection + residual -> xmid (fp32)
                    with ExitStack() as sProj:
                        xqps = sProj.enter_context(
                            tc.tile_pool(name="xqpp", bufs=1)).tile(
                            [128, NS, E], f32)
                        for s in range(NS):
                            nc.sync.dma_start(
                                xqps[:, s, :], xqp[s * 128:(s + 1) * 128, :])
                        pw_sb = sProj.enter_context(
                            tc.tile_pool(name="pwp", bufs=1)).tile(
                            [128, NP, E], bf16)
                        nc.sync.dma_start(pw_sb[:], projw.rearrange(
                            "p (m e) -> p m e", m=NP))
                        pps = sProj.enter_context(
                            tc.tile_pool(name="pps", bufs=3, space="PSUM"))
                        for qm in range(NS):
                            ps = pps.tile([128, 1024], f32)
                            for ec in range(2):
                                for pk in range(NP):
                                    nc.tensor.matmul(
                                        ps[:, ec * 512:(ec + 1) * 512],
                                        lhsT=oT[:, pk, qm * 128:(qm + 1) * 128],
                                        rhs=pw_sb[:, pk, ec * 512:(ec + 1) * 512],
                                        start=(pk == 0), stop=(pk == NP - 1))
                            nc.vector.tensor_add(
                                xmid[:, qm, :], ps[:], xqps[:, qm, :])

            # ---------- scope D: LN2 + FFN ----------
            with ExitStack() as sD:
                ln2T = sD.enter_context(tc.tile_pool(name="ln2p", bufs=1)).tile(
                    [128, ET, M], bf16)
                w2_sb = sD.enter_context(tc.tile_pool(name="w2p", bufs=1)).tile(
                    [128, FT, E], bf16)
                nc.sync.dma_start(w2_sb[:], w2.rearrange(
                    "p (ft e) -> p ft e", ft=FT))

                with ExitStack() as sLN2:
                    statp2 = sLN2.enter_context(tc.tile_pool(name="statp2", bufs=6))
                    lnstage2 = sLN2.enter_context(
                        tc.tile_pool(name="lnstage2", bufs=3))
                    tpsum2 = sLN2.enter_context(
                        tc.tile_pool(name="tpsum2", bufs=3, space="PSUM"))
                    for qm in range(NS):
                        ln_chunk(xmid[:, qm, :], ln2T, qm * 128, statp2,
                                 lnstage2, tpsum2, qm)
                        nc.vector.tensor_add(xmid[:, qm, :], xmid[:, qm, :],
                                             bf2[:])

                rtp = sD.enter_context(tc.tile_pool(name="rtp", bufs=1))
                w1p = sD.enter_context(tc.tile_pool(name="w1p", bufs=4))
                zps = sD.enter_context(
                    tc.tile_pool(name="zps", bufs=2, space="PSUM"))
                ops = sD.enter_context(
                    tc.tile_pool(name="ops", bufs=2, space="PSUM"))
                outp = sD.enter_context(tc.tile_pool(name="outp", bufs=3))

                for half in range(2):
                    rT = rtp.tile([128, FT, 512], bf16, tag="rT")
                    for fm in range(FT):
                        w1f = w1p.tile([128, ET, 128], bf16)
                        nc.sync.dma_start(
                            w1f[:], w1.rearrange("p (fm et f) -> p fm et f",
                                                 fm=FT, et=ET)[:, fm])
                        zp = zps.tile([128, 512], f32)
                        for et in range(ET):
                            nc.tensor.matmul(
                                zp[:],
                                lhsT=w1f[:, et, :],
                                rhs=ln2T[:, et, half * 512:(half + 1) * 512],
                                start=(et == 0), stop=(et == ET - 1))
                        nc.scalar.activation(rT[:, fm, :], zp[:], AF.Relu,
                                             bias=b1[:, fm:fm + 1])
                    for qq in range(4):
                        qm = half * 4 + qq
                        ot = outp.tile([128, E], f32)
                        op = ops.tile([128, 1024], f32)
                        for ec in range(2):
                            for fk in range(FT):
                                nc.tensor.matmul(
                                    op[:, ec * 512:(ec + 1) * 512],
                                    lhsT=rT[:, fk, qq * 128:(qq + 1) * 128],
                                    rhs=w2_sb[:, fk, ec * 512:(ec + 1) * 512],
                                    start=(fk == 0), stop=(fk == FT - 1))
                        nc.vector.tensor_add(ot[:], op[:], xmid[:, qm, :])
                        nc.scalar.dma_start(out[qm * 128:(qm + 1) * 128, :], ot[:])

    nc.compile()
    _CACHE[key] = nc
    return nc


def _prep_inputs(x, wq, wk, wv, proj_w, proj_b, g1, beta1, g2, beta2, w1, bf1,
                 w2, bf2):
    """Host-side sharding + weight folding. Returns list of 8 in_maps."""
    f32 = np.float32
    x = np.asarray(x, f32)
    scale = float(E) ** -0.5

    Wq = np.asarray(wq, f32).transpose(1, 0, 2).reshape(E, H * D) * scale
    Wk = np.asarray(wk, f32).transpose(1, 0, 2).reshape(E, H * D)
    Wv = np.asarray(wv, f32).transpose(1, 0, 2).reshape(E, H * D)
    g1 = np.asarray(g1, f32)
    beta1 = np.asarray(beta1, f32)
    g2 = np.asarray(g2, f32)
    beta2 = np.asarray(beta2, f32)
    w1 = np.asarray(w1, f32)
    w2 = np.asarray(w2, f32)
    bf1 = np.asarray(bf1, f32)
    bf2 = np.asarray(bf2, f32)
    proj_w = np.asarray(proj_w, f32)
    proj_b = np.asarray(proj_b, f32)

    def sb_layout(w, ntile):
        n = w.shape[1]
        return np.ascontiguousarray(
            w.reshape(ntile, 128, n).transpose(1, 0, 2).reshape(128, ntile * n))

    wq_b = sb_layout((Wq * g1[:, None]).astype(BF16), ET)
    wk_b = sb_layout((Wk * g1[:, None]).astype(BF16), ET)
    wv_b = sb_layout((Wv * g1[:, None]).astype(BF16), ET)
    qbias = beta1 @ Wq
    kbias = beta1 @ Wk
    vbias = beta1 @ Wv
    w1_b = np.ascontiguousarray(
        (w1 * g2[:, None]).astype(BF16)
        .reshape(ET, 128, FT, 128).transpose(1, 2, 0, 3)
        .reshape(128, FT * ET * 128))
    b1v = bf1 + beta2 @ w1
    w2_b = sb_layout(w2.astype(BF16), FT)
    projw_b = sb_layout(proj_w.astype(BF16), NP)

    qb = np.ascontiguousarray(qbias.reshape(ET, 128).T, f32)
    kb = np.ascontiguousarray(kbias.reshape(ET, 128).T, f32)
    vb = np.ascontiguousarray(np.broadcast_to(vbias, (128, H * D)), f32)
    b1m = np.ascontiguousarray(b1v.reshape(FT, 128).T, f32)
    bf2m = np.ascontiguousarray(np.broadcast_to(bf2, (128, E)), f32)

    tri = np.triu(np.ones((128, 128), f32))  # [k_row, q_col]: 1 iff k <= q
    onesm = np.ones((128, 128), f32)
    zerosm = np.zeros((128, 128), f32)
    mP = {0: zerosm, 1: onesm}

    in_maps = []
    for c in range(NCORES):
        b, hpar = c // 2, c % 2
        xb = x[b].reshape(TK, 128, E)
        # permute: own-parity blocks first, other-parity after
        perm = list(range(hpar, TK, 2)) + list(range(1 - hpar, TK, 2))
        xp = np.ascontiguousarray(xb[perm].reshape(T, E), f32)
        xq = xp[:M]
        in_maps.append({
            "x_full": xp,
            "xqp": np.ascontiguousarray(xq + proj_b[None, :].astype(f32)),
            "wq": wq_b, "wk": wk_b, "wv": wv_b,
            "projw": projw_b, "w1": w1_b, "w2": w2_b,
            "qb": qb, "kb": kb, "vb": vb, "b1": b1m, "bf2b": bf2m,
            "maskT": np.ascontiguousarray(
                np.tile(tri, (1, 2))).astype(BF16),
            "maskP": np.ascontiguousarray(
                np.tile(mP[hpar], (1, 2))).astype(BF16),
        })
    return in_maps


def _run(inputs, trace=False):
    from concourse.bass_utils import run_bass_kernel_spmd
    nc = _build()
    in_maps = _prep_inputs(**inputs)
    res = run_bass_kernel_spmd(nc, in_maps, core_ids=list(range(NCORES)),
                               trace=trace)
    full = np.empty((B, T, E), np.float32)
    for c in range(NCORES):
        b, hpar = c // 2, c % 2
        full[b].reshape(TK, 128, E)[hpar::2] = (
            res.results[c]["out"].reshape(NS, 128, E))
    return full, res


def kernel(**inputs) -> np.ndarray:
    out, _ = _run(inputs, trace=False)
    return out
